# revision 4
# baseline (speedup 1.0000x reference)
"""Multi-head attention (B=4, S=2048, H=1024, 16 heads) on 8 Trainium2 NeuronCores.

Sharding: data-parallel over (batch, seq-half) -> 8 fully independent shards, no
collectives. Each core computes attention for 1024 query tokens of one batch
element; K/V are computed redundantly for the full 2048-token sequence of that
batch (+25% projection flops buys zero cross-core communication).

Per-core scheme (feature-major "transposed" layout):
  - qT/kT feature-major [H, tokens]; each 128-row chunk = one head pair
  - scores transposed S_T[j,i]; the two heads of a chunk run CONCURRENTLY on
    the PE via tile_position row packing inside a tc.tile_critical() pair
    (K=64 solo streams at half rate: 492ns vs 143ns/mm measured paired)
  - softmax: exp on ACT from PSUM in [128,1024] tiles ((N+352)/1.2ns each);
    no max subtraction needed (|s|/8 < 8 << 88, fp32 exp exact enough);
    denominators ride along as a 65th ones-column of v in the PV matmul
  - PV: stationary = probs_T chunk, moving = v_ext; both heads of a pair
    accumulate into one [128,130] PSUM tile; divide by denominator
    (per-partition scalar on DVE), PE-transpose to feature-major (col
    tile_position places odd head at partitions 64..127), output projection
    accumulates all 16 heads in PSUM.
  - biases: bq/bk per-partition adds on the PSUM->SBUF copy (DVE); bv via an
    augmented ones-row matmul; bo added on host (exact, linear).
"""

import numpy as np
import ml_dtypes
from contextlib import ExitStack

import concourse.tile as tile
from concourse import bacc, mybir
from concourse.bass_utils import run_bass_kernel_spmd
from concourse.masks import make_identity

B, S, H, NH, HD = 4, 2048, 1024, 16, 64
T = 1024          # query tokens per core
TK = 2048         # key tokens per core
NCORES = 8
FC = H // 128     # 8 feature chunks (= head pairs)
KC = TK // 128    # 16 key-token chunks
F32 = mybir.dt.float32
BF16 = mybir.dt.bfloat16
BFNP = ml_dtypes.bfloat16
Act = mybir.ActivationFunctionType
Alu = mybir.AluOpType

_CACHE = {}


def _emit(ctx: ExitStack, tc, d):
    nc = tc.nc
    xT, wqT, wkT, wvT = d["xT"], d["wqT"], d["wkT"], d["wvT"]
    bvrow, woT, bqc, bkc, out = d["bvrow"], d["woT"], d["bqc"], d["bkc"], d["out"]

    pers = ctx.enter_context(tc.tile_pool(name="pers", bufs=1))
    ident = pers.tile([128, 128], BF16, tag="ident")
    make_identity(nc, ident[:])

    kT_t = [pers.tile([128, TK], BF16, tag="kT", bufs=FC, name=f"kT{i}") for i in range(FC)]
    qT_t = [pers.tile([128, T], BF16, tag="qT", bufs=FC, name=f"qT{i}") for i in range(FC)]
    v_t = [pers.tile([128, NH * (HD + 1)], BF16, tag="vsb", bufs=KC, name=f"v{i}") for i in range(KC)]
    oT_t = [pers.tile([128, T], BF16, tag="oT", bufs=FC, name=f"oT{i}") for i in range(FC)]
    bq_sb = pers.tile([128, FC], F32, tag="bq")
    bk_sb = pers.tile([128, FC], F32, tag="bk")
    nc.sync.dma_start(bq_sb[:], bqc[:, :])
    nc.sync.dma_start(bk_sb[:], bkc[:, :])

    # PSUM: pp (2 banks) + pst (4 banks) coexist; po/pt (4 banks) open after
    # pp closes -> never exceeds 8 banks while letting attention overlap ph1.
    pst = ctx.enter_context(tc.tile_pool(name="pst", bufs=2, space="PSUM"))

    # SBUF pool stack (LIFO release): pin -> pvin -> pqk (released in reverse)
    stack_x = ExitStack()
    stack_v = ExitStack()
    stack_qk = ExitStack()

    pin = stack_x.enter_context(tc.tile_pool(name="pin", bufs=1))
    x_t = [pin.tile([128, TK], BF16, tag="xin", bufs=FC, name=f"x{i}") for i in range(FC)]
    ones_r = pin.tile([1, TK], BF16, tag="ones")
    pv_in = stack_v.enter_context(tc.tile_pool(name="pvin", bufs=1))
    wv_t = [pv_in.tile([128, H], BF16, tag="wvin", bufs=FC, name=f"wv{i}") for i in range(FC)]
    bv_r = pv_in.tile([1, H], BF16, tag="bvr")
    pqk = stack_qk.enter_context(tc.tile_pool(name="pqk", bufs=1))
    wq_t = [pqk.tile([128, H], BF16, tag="wqin", bufs=FC, name=f"wq{i}") for i in range(FC)]
    wk_t = [pqk.tile([128, H], BF16, tag="wkin", bufs=FC, name=f"wk{i}") for i in range(FC)]

    for fc in range(FC):
        nc.sync.dma_start(x_t[fc][:], xT[fc * 128:(fc + 1) * 128, :])
        nc.sync.dma_start(wq_t[fc][:], wqT[fc * 128:(fc + 1) * 128, :])
        nc.sync.dma_start(wk_t[fc][:], wkT[fc * 128:(fc + 1) * 128, :])
        nc.sync.dma_start(wv_t[fc][:], wvT[fc * 128:(fc + 1) * 128, :])
    nc.sync.dma_start(bv_r[:1, :], bvrow[:, :])
    nc.gpsimd.memset(ones_r[:1, :], 1.0)

    pp = ExitStack()
    ppool = pp.enter_context(tc.tile_pool(name="ppool", bufs=2, space="PSUM"))

    # ---- qT / kT per head pair (so attention on pair hp unblocks early) ----
    for fc in range(FC):
        for th in range(T // 512):
            ps = ppool.tile([128, 512], F32, tag="pp", name=f"psq{fc}_{th}")
            for c in range(FC):
                nc.tensor.matmul(
                    ps[:], lhsT=wq_t[c][:, fc * 128:(fc + 1) * 128],
                    rhs=x_t[c][:, th * 512:(th + 1) * 512],
                    start=(c == 0), stop=(c == FC - 1))
            nc.vector.tensor_scalar(qT_t[fc][:, th * 512:(th + 1) * 512], ps[:],
                                    bq_sb[:, fc:fc + 1], None, Alu.add)
        for th in range(TK // 512):
            ps = ppool.tile([128, 512], F32, tag="pp", name=f"psk{fc}_{th}")
            for c in range(FC):
                nc.tensor.matmul(
                    ps[:], lhsT=wk_t[c][:, fc * 128:(fc + 1) * 128],
                    rhs=x_t[c][:, th * 512:(th + 1) * 512],
                    start=(c == 0), stop=(c == FC - 1))
            nc.vector.tensor_scalar(kT_t[fc][:, th * 512:(th + 1) * 512], ps[:],
                                    bk_sb[:, fc:fc + 1], None, Alu.add)
    stack_qk.close()

    # ---- v token-major (+bias via ones-row), per-head 64 cols + ones col ----
    for kc in range(KC):
        nc.gpsimd.memset(v_t[kc][:], 1.0)
        for mh in range(H // 512):
            ps = ppool.tile([128, 512], F32, tag="pp", name=f"psv{kc}_{mh}")
            for c in range(FC):
                nc.tensor.matmul(
                    ps[:], lhsT=x_t[c][:, kc * 128:(kc + 1) * 128],
                    rhs=wv_t[c][:, mh * 512:(mh + 1) * 512],
                    start=(c == 0), stop=False)
            nc.tensor.matmul(
                ps[:], lhsT=ones_r[:1, kc * 128:(kc + 1) * 128],
                rhs=bv_r[:1, mh * 512:(mh + 1) * 512],
                start=False, stop=True)
            for hh in range(512 // HD):
                h = mh * 8 + hh
                nc.vector.tensor_copy(
                    v_t[kc][:, h * (HD + 1):h * (HD + 1) + HD],
                    ps[:, hh * HD:(hh + 1) * HD])
    pp.close()
    stack_v.close()
    stack_x.close()

    # woT loads into space freed by the input pools (only needed for phase 3)
    wo_t = [pers.tile([128, H], BF16, tag="woT", bufs=FC, name=f"wo{i}") for i in range(FC)]
    for fc in range(FC):
        nc.sync.dma_start(wo_t[fc][:], woT[fc * 128:(fc + 1) * 128, :])

    # ---------------- attention ----------------
    with ExitStack() as ph2:
        po = ph2.enter_context(tc.tile_pool(name="po", bufs=2, space="PSUM"))
        pt = ph2.enter_context(tc.tile_pool(name="pt", bufs=2, space="PSUM"))
        prb = ph2.enter_context(tc.tile_pool(name="prb", bufs=22))
        sml = ph2.enter_context(tc.tile_pool(name="sml", bufs=4))

        for hp in range(FC):
            for ih in range(T // 512):
                i0 = ih * 512
                # scores + exp: [128,1024] tiles hold jc-pairs; two heads of the
                # pair run concurrently via tile_position row packing.
                ptiles = [[None] * (KC // 2), [None] * (KC // 2)]
                for jcp in range(KC // 2):
                    st = [pst.tile([128, 1024], F32, tag="st", name=f"st{hp}_{ih}_{jcp}_{hh}")
                          for hh in range(2)]
                    for sub in range(2):
                        jc = 2 * jcp + sub
                        with tc.tile_critical():
                            nc.tensor.matmul(
                                st[0][:, sub * 512:(sub + 1) * 512],
                                lhsT=kT_t[hp][0:64, jc * 128:(jc + 1) * 128],
                                rhs=qT_t[hp][0:64, i0:i0 + 512],
                                start=True, stop=True, tile_position=(0, 0))
                            nc.tensor.matmul(
                                st[1][:, sub * 512:(sub + 1) * 512],
                                lhsT=kT_t[hp][64:128, jc * 128:(jc + 1) * 128],
                                rhs=qT_t[hp][64:128, i0:i0 + 512],
                                start=True, stop=True, tile_position=(64, 0))
                    for hh in range(2):
                        pr = prb.tile([128, 1024], BF16, tag="pr", bufs=22,
                                      name=f"pr{hp}_{ih}_{jcp}_{hh}")
                        nc.scalar.activation(pr[:], st[hh][:], Act.Exp, scale=0.125)
                        ptiles[hh][jcp] = pr
                # PV + divide + transpose back to feature-major
                for ic in range(4):
                    oe = po.tile([128, 2 * (HD + 1)], F32, tag="po", name=f"oe{hp}_{ih}_{ic}")
                    for hh in range(2):
                        h = 2 * hp + hh
                        ob = hh * (HD + 1)
                        for jc in range(KC):
                            nc.tensor.matmul(
                                oe[:, ob:ob + HD + 1],
                                lhsT=ptiles[hh][jc // 2][:, (jc % 2) * 512 + ic * 128:(jc % 2) * 512 + ic * 128 + 128],
                                rhs=v_t[jc][:, h * (HD + 1):(h + 1) * (HD + 1)],
                                start=(jc == 0), stop=(jc == KC - 1))
                    ptile = pt.tile([128, 128], BF16, tag="pt", name=f"pt{hp}_{ih}_{ic}")
                    for hh in range(2):
                        ob = hh * (HD + 1)
                        rec = sml.tile([128, 1], F32, tag="rec", name=f"rec{hp}_{ih}_{ic}_{hh}")
                        nc.vector.reciprocal(rec[:], oe[:, ob + HD:ob + HD + 1])
                        od = sml.tile([128, HD], BF16, tag="od", name=f"od{hp}_{ih}_{ic}_{hh}")
                        nc.vector.tensor_scalar(od[:], oe[:, ob:ob + HD], rec[:], None, Alu.mult)
                        nc.tensor.transpose(ptile[hh * 64:(hh + 1) * 64, :], od[:],
                                            ident[:], tile_position=(0, hh * 64))
                    nc.vector.tensor_copy(
                        oT_t[hp][:, i0 + ic * 128:i0 + (ic + 1) * 128], ptile[:])

    # ---------------- output projection ----------------
    with ExitStack() as ph3:
        pf = ph3.enter_context(tc.tile_pool(name="pf", bufs=2, space="PSUM"))
        fout = ph3.enter_context(tc.tile_pool(name="fout", bufs=2))
        for tcn in range(T // 128):
            fo = fout.tile([128, H], F32, tag="fo", name=f"fo{tcn}")
            for mh in range(H // 512):
                psf = pf.tile([128, 512], F32, tag="pf", name=f"pf{tcn}_{mh}")
                for fc in range(FC):
                    nc.tensor.matmul(
                        psf[:], lhsT=oT_t[fc][:, tcn * 128:(tcn + 1) * 128],
                        rhs=wo_t[fc][:, mh * 512:(mh + 1) * 512],
                        start=(fc == 0), stop=(fc == FC - 1))
                nc.vector.tensor_copy(fo[:, mh * 512:(mh + 1) * 512], psf[:])
            nc.sync.dma_start(out[tcn * 128:(tcn + 1) * 128, :], fo[:])


def _build():
    nc = bacc.Bacc("TRN2", target_bir_lowering=False, debug=False, enable_asserts=True)
    d = {}
    d["xT"] = nc.dram_tensor("xT", [H, TK], BF16, kind="ExternalInput").ap()
    d["wqT"] = nc.dram_tensor("wqT", [H, H], BF16, kind="ExternalInput").ap()
    d["wkT"] = nc.dram_tensor("wkT", [H, H], BF16, kind="ExternalInput").ap()
    d["wvT"] = nc.dram_tensor("wvT", [H, H], BF16, kind="ExternalInput").ap()
    d["bvrow"] = nc.dram_tensor("bvrow", [1, H], BF16, kind="ExternalInput").ap()
    d["woT"] = nc.dram_tensor("woT", [H, H], BF16, kind="ExternalInput").ap()
    d["bqc"] = nc.dram_tensor("bqc", [128, FC], F32, kind="ExternalInput").ap()
    d["bkc"] = nc.dram_tensor("bkc", [128, FC], F32, kind="ExternalInput").ap()
    d["out"] = nc.dram_tensor("out", [T, H], F32, kind="ExternalOutput").ap()
    with tile.TileContext(nc) as tc:
        with ExitStack() as ctx:
            _emit(ctx, tc, d)
    nc.compile()
    return nc


def get_nc():
    if "nc" not in _CACHE:
        _CACHE["nc"] = _build()
    return _CACHE["nc"]


def make_in_maps(inputs):
    x = np.asarray(inputs["hidden_states"], dtype=np.float32)
    wq = np.asarray(inputs["wq"], dtype=np.float32)
    wk = np.asarray(inputs["wk"], dtype=np.float32)
    wv = np.asarray(inputs["wv"], dtype=np.float32)
    wo = np.asarray(inputs["wo"], dtype=np.float32)
    bq = np.asarray(inputs["bq"], dtype=np.float32)
    bk = np.asarray(inputs["bk"], dtype=np.float32)
    bv = np.asarray(inputs["bv"], dtype=np.float32)

    wqT = np.ascontiguousarray(wq.T).astype(BFNP)
    wkT = np.ascontiguousarray(wk.T).astype(BFNP)
    wvT = np.ascontiguousarray(wv.T).astype(BFNP)
    woT = np.ascontiguousarray(wo.T).astype(BFNP)
    bvrow = bv.reshape(1, H).astype(BFNP)
    # feature-major bias chunks: partition p, col fc -> bias[fc*128 + p]
    bqc = np.ascontiguousarray(bq.reshape(FC, 128).T)
    bkc = np.ascontiguousarray(bk.reshape(FC, 128).T)

    in_maps = []
    for c in range(NCORES):
        b, hf = divmod(c, 2)
        xb = x[b]
        # roll so this core's query tokens are tokens [0:T); key order is
        # irrelevant to attention (softmax/PV sum over keys).
        rolled = np.concatenate([xb[hf * T:], xb[:hf * T]], axis=0) if hf else xb
        xT = np.ascontiguousarray(rolled.T).astype(BFNP)
        in_maps.append({
            "xT": xT, "wqT": wqT, "wkT": wkT, "wvT": wvT,
            "bvrow": bvrow, "woT": woT, "bqc": bqc, "bkc": bkc,
        })
    return in_maps


def kernel(**inputs):
    nc = get_nc()
    in_maps = make_in_maps(inputs)
    res = run_bass_kernel_spmd(nc, in_maps, core_ids=list(range(NCORES)))
    bo = np.asarray(inputs["bo"], dtype=np.float32)
    out = np.empty((B, S, H), dtype=np.float32)
    for c in range(NCORES):
        b, hf = divmod(c, 2)
        out[b, hf * T:(hf + 1) * T, :] = res.results[c]["out"]
    out += bo[None, None, :]
    return out


# revision 5
# speedup vs baseline: 1.7749x; 1.7749x over previous
"""Multi-head attention (B=4, S=2048, H=1024, 16 heads) on 8 Trainium2 NeuronCores.

Sharding: data-parallel over (batch, seq-half) -> 8 fully independent shards, no
collectives. Each core computes attention for 1024 query tokens of one batch
element; K/V are computed redundantly for the full 2048-token sequence of that
batch (+25% projection flops buys zero cross-core communication).

Per-core scheme (feature-major "transposed" layout):
  - qT/kT feature-major [H, tokens]; each 128-row chunk = one head pair
  - scores transposed S_T[j,i]; the two heads of a chunk run CONCURRENTLY on
    the PE via tile_position row packing inside a tc.tile_critical() pair
    (K=64 solo streams at half rate: 492ns vs 143ns/mm measured paired)
  - softmax: exp on ACT from PSUM in [128,1024] tiles ((N+352)/1.2ns each);
    no max subtraction needed (|s|/8 < 8 << 88, fp32 exp exact enough);
    denominators ride along as a 65th ones-column of v in the PV matmul
  - PV: stationary = probs_T chunk, moving = v_ext; both heads of a pair
    accumulate into one [128,130] PSUM tile; divide by denominator
    (per-partition scalar on DVE), PE-transpose to feature-major (col
    tile_position places odd head at partitions 64..127), output projection
    accumulates all 16 heads in PSUM.
  - biases: bq/bk per-partition adds on the PSUM->SBUF copy (DVE); bv via an
    augmented ones-row matmul; bo added on host (exact, linear).
"""

import numpy as np
import ml_dtypes
from contextlib import ExitStack

import concourse.tile as tile
from concourse import bacc, mybir
from concourse.bass_utils import run_bass_kernel_spmd
from concourse.masks import make_identity

B, S, H, NH, HD = 4, 2048, 1024, 16, 64
T = 1024          # query tokens per core
TK = 2048         # key tokens per core
NCORES = 8
FC = H // 128     # 8 feature chunks (= head pairs)
KC = TK // 128    # 16 key-token chunks
F32 = mybir.dt.float32
BF16 = mybir.dt.bfloat16
BFNP = ml_dtypes.bfloat16
Act = mybir.ActivationFunctionType
Alu = mybir.AluOpType

_CACHE = {}


def _emit(ctx: ExitStack, tc, d):
    nc = tc.nc
    xT, wqT, wkT, wvT = d["xT"], d["wqT"], d["wkT"], d["wvT"]
    bvrow, woT, bqc, bkc, out = d["bvrow"], d["woT"], d["bqc"], d["bkc"], d["out"]

    pers = ctx.enter_context(tc.tile_pool(name="pers", bufs=1))
    ident = pers.tile([128, 128], BF16, tag="ident")
    make_identity(nc, ident[:])

    kT_t = [pers.tile([128, TK], BF16, tag="kT", bufs=FC, name=f"kT{i}") for i in range(FC)]
    qT_t = [pers.tile([128, T], BF16, tag="qT", bufs=FC, name=f"qT{i}") for i in range(FC)]
    v_t = [pers.tile([128, NH * (HD + 1)], BF16, tag="vsb", bufs=KC, name=f"v{i}") for i in range(KC)]
    oT_t = [pers.tile([128, T], BF16, tag="oT", bufs=FC, name=f"oT{i}") for i in range(FC)]
    bq_sb = pers.tile([128, FC], F32, tag="bq")
    bk_sb = pers.tile([128, FC], F32, tag="bk")
    nc.sync.dma_start(bq_sb[:], bqc[:, :])
    nc.sync.dma_start(bk_sb[:], bkc[:, :])

    # PSUM: pp (2 banks) + pst (4 banks) coexist; po/pt (4 banks) open after
    # pp closes -> never exceeds 8 banks while letting attention overlap ph1.
    pst = ctx.enter_context(tc.tile_pool(name="pst", bufs=2, space="PSUM"))

    # SBUF pool stack (LIFO release): pin -> pvin -> pqk (released in reverse)
    stack_x = ExitStack()
    stack_v = ExitStack()
    stack_qk = ExitStack()

    pin = stack_x.enter_context(tc.tile_pool(name="pin", bufs=1))
    x_t = [pin.tile([128, TK], BF16, tag="xin", bufs=FC, name=f"x{i}") for i in range(FC)]
    ones_r = pin.tile([1, TK], BF16, tag="ones")
    pv_in = stack_v.enter_context(tc.tile_pool(name="pvin", bufs=1))
    wv_t = [pv_in.tile([128, H], BF16, tag="wvin", bufs=FC, name=f"wv{i}") for i in range(FC)]
    bv_r = pv_in.tile([1, H], BF16, tag="bvr")
    pqk = stack_qk.enter_context(tc.tile_pool(name="pqk", bufs=1))
    wq_t = [pqk.tile([128, H], BF16, tag="wqin", bufs=FC, name=f"wq{i}") for i in range(FC)]
    wk_t = [pqk.tile([128, H], BF16, tag="wkin", bufs=FC, name=f"wk{i}") for i in range(FC)]

    for fc in range(FC):
        nc.sync.dma_start(x_t[fc][:], xT[fc * 128:(fc + 1) * 128, :])
        nc.sync.dma_start(wq_t[fc][:], wqT[fc * 128:(fc + 1) * 128, :])
        nc.sync.dma_start(wk_t[fc][:], wkT[fc * 128:(fc + 1) * 128, :])
        nc.sync.dma_start(wv_t[fc][:], wvT[fc * 128:(fc + 1) * 128, :])
    nc.sync.dma_start(bv_r[:1, :], bvrow[:, :])
    nc.gpsimd.memset(ones_r[:1, :], 1.0)

    pp = ExitStack()
    ppool = pp.enter_context(tc.tile_pool(name="ppool", bufs=2, space="PSUM"))

    # ---- qT / kT per head pair (so attention on pair hp unblocks early) ----
    for fc in range(FC):
        for th in range(T // 512):
            ps = ppool.tile([128, 512], F32, tag="pp", name=f"psq{fc}_{th}")
            for c in range(FC):
                nc.tensor.matmul(
                    ps[:], lhsT=wq_t[c][:, fc * 128:(fc + 1) * 128],
                    rhs=x_t[c][:, th * 512:(th + 1) * 512],
                    start=(c == 0), stop=(c == FC - 1))
            nc.vector.tensor_scalar(qT_t[fc][:, th * 512:(th + 1) * 512], ps[:],
                                    bq_sb[:, fc:fc + 1], None, Alu.add)
        for th in range(TK // 512):
            ps = ppool.tile([128, 512], F32, tag="pp", name=f"psk{fc}_{th}")
            for c in range(FC):
                nc.tensor.matmul(
                    ps[:], lhsT=wk_t[c][:, fc * 128:(fc + 1) * 128],
                    rhs=x_t[c][:, th * 512:(th + 1) * 512],
                    start=(c == 0), stop=(c == FC - 1))
            nc.vector.tensor_scalar(kT_t[fc][:, th * 512:(th + 1) * 512], ps[:],
                                    bk_sb[:, fc:fc + 1], None, Alu.add)
    stack_qk.close()

    # ---- v token-major (+bias via ones-row), per-head 64 cols + ones col ----
    for kc in range(KC):
        nc.gpsimd.memset(v_t[kc][:], 1.0)
        for mh in range(H // 512):
            ps = ppool.tile([128, 512], F32, tag="pp", name=f"psv{kc}_{mh}")
            for c in range(FC):
                nc.tensor.matmul(
                    ps[:], lhsT=x_t[c][:, kc * 128:(kc + 1) * 128],
                    rhs=wv_t[c][:, mh * 512:(mh + 1) * 512],
                    start=(c == 0), stop=False)
            nc.tensor.matmul(
                ps[:], lhsT=ones_r[:1, kc * 128:(kc + 1) * 128],
                rhs=bv_r[:1, mh * 512:(mh + 1) * 512],
                start=False, stop=True)
            for hh in range(512 // HD):
                h = mh * 8 + hh
                nc.vector.tensor_copy(
                    v_t[kc][:, h * (HD + 1):h * (HD + 1) + HD],
                    ps[:, hh * HD:(hh + 1) * HD])
    pp.close()
    stack_v.close()
    stack_x.close()

    # woT loads into space freed by the input pools (only needed for phase 3)
    wo_t = [pers.tile([128, H], BF16, tag="woT", bufs=FC, name=f"wo{i}") for i in range(FC)]
    for fc in range(FC):
        nc.sync.dma_start(wo_t[fc][:], woT[fc * 128:(fc + 1) * 128, :])

    # ---------------- attention ----------------
    with ExitStack() as ph2:
        po = ph2.enter_context(tc.tile_pool(name="po", bufs=2, space="PSUM"))
        pt = ph2.enter_context(tc.tile_pool(name="pt", bufs=2, space="PSUM"))
        prb = ph2.enter_context(tc.tile_pool(name="prb", bufs=22))
        sml = ph2.enter_context(tc.tile_pool(name="sml", bufs=4))

        for hp in range(FC):
            for ih in range(T // 512):
                i0 = ih * 512
                # scores + exp: one [128,1024] tile per jc holds BOTH heads
                # [h0 512i | h1 512i] -> the paired tile_position matmuls are
                # gated by a single slot and issue back-to-back (pair
                # concurrency on the PE), and one exp drains the whole tile.
                ptiles = [None] * KC
                for jc in range(KC):
                    st2 = pst.tile([128, 1024], F32, tag="st", name=f"st{hp}_{ih}_{jc}")
                    nc.tensor.matmul(
                        st2[:, 0:512],
                        lhsT=kT_t[hp][0:64, jc * 128:(jc + 1) * 128],
                        rhs=qT_t[hp][0:64, i0:i0 + 512],
                        start=True, stop=True, tile_position=(0, 0))
                    nc.tensor.matmul(
                        st2[:, 512:1024],
                        lhsT=kT_t[hp][64:128, jc * 128:(jc + 1) * 128],
                        rhs=qT_t[hp][64:128, i0:i0 + 512],
                        start=True, stop=True, tile_position=(64, 0))
                    pr = prb.tile([128, 1024], BF16, tag="pr", bufs=22,
                                  name=f"pr{hp}_{ih}_{jc}")
                    nc.scalar.activation(pr[:], st2[:], Act.Exp, scale=0.125)
                    ptiles[jc] = pr
                # PV + divide + transpose back to feature-major
                for ic in range(4):
                    oe = po.tile([128, 2 * (HD + 1)], F32, tag="po", name=f"oe{hp}_{ih}_{ic}")
                    for hh in range(2):
                        h = 2 * hp + hh
                        ob = hh * (HD + 1)
                        for jc in range(KC):
                            nc.tensor.matmul(
                                oe[:, ob:ob + HD + 1],
                                lhsT=ptiles[jc][:, hh * 512 + ic * 128:hh * 512 + ic * 128 + 128],
                                rhs=v_t[jc][:, h * (HD + 1):(h + 1) * (HD + 1)],
                                start=(jc == 0), stop=(jc == KC - 1))
                    ptile = pt.tile([128, 128], BF16, tag="pt", name=f"pt{hp}_{ih}_{ic}")
                    for hh in range(2):
                        ob = hh * (HD + 1)
                        rec = sml.tile([128, 1], F32, tag="rec", name=f"rec{hp}_{ih}_{ic}_{hh}")
                        nc.vector.reciprocal(rec[:], oe[:, ob + HD:ob + HD + 1])
                        od = sml.tile([128, HD], BF16, tag="od", name=f"od{hp}_{ih}_{ic}_{hh}")
                        nc.vector.tensor_scalar(od[:], oe[:, ob:ob + HD], rec[:], None, Alu.mult)
                        nc.tensor.transpose(ptile[hh * 64:(hh + 1) * 64, :], od[:],
                                            ident[:], tile_position=(0, hh * 64))
                    nc.vector.tensor_copy(
                        oT_t[hp][:, i0 + ic * 128:i0 + (ic + 1) * 128], ptile[:])

    # ---------------- output projection ----------------
    with ExitStack() as ph3:
        pf = ph3.enter_context(tc.tile_pool(name="pf", bufs=2, space="PSUM"))
        fout = ph3.enter_context(tc.tile_pool(name="fout", bufs=2))
        for tcn in range(T // 128):
            fo = fout.tile([128, H], F32, tag="fo", name=f"fo{tcn}")
            for mh in range(H // 512):
                psf = pf.tile([128, 512], F32, tag="pf", name=f"pf{tcn}_{mh}")
                for fc in range(FC):
                    nc.tensor.matmul(
                        psf[:], lhsT=oT_t[fc][:, tcn * 128:(tcn + 1) * 128],
                        rhs=wo_t[fc][:, mh * 512:(mh + 1) * 512],
                        start=(fc == 0), stop=(fc == FC - 1))
                nc.vector.tensor_copy(fo[:, mh * 512:(mh + 1) * 512], psf[:])
            nc.sync.dma_start(out[tcn * 128:(tcn + 1) * 128, :], fo[:])


def _build():
    nc = bacc.Bacc("TRN2", target_bir_lowering=False, debug=False, enable_asserts=True)
    d = {}
    d["xT"] = nc.dram_tensor("xT", [H, TK], BF16, kind="ExternalInput").ap()
    d["wqT"] = nc.dram_tensor("wqT", [H, H], BF16, kind="ExternalInput").ap()
    d["wkT"] = nc.dram_tensor("wkT", [H, H], BF16, kind="ExternalInput").ap()
    d["wvT"] = nc.dram_tensor("wvT", [H, H], BF16, kind="ExternalInput").ap()
    d["bvrow"] = nc.dram_tensor("bvrow", [1, H], BF16, kind="ExternalInput").ap()
    d["woT"] = nc.dram_tensor("woT", [H, H], BF16, kind="ExternalInput").ap()
    d["bqc"] = nc.dram_tensor("bqc", [128, FC], F32, kind="ExternalInput").ap()
    d["bkc"] = nc.dram_tensor("bkc", [128, FC], F32, kind="ExternalInput").ap()
    d["out"] = nc.dram_tensor("out", [T, H], F32, kind="ExternalOutput").ap()
    with tile.TileContext(nc) as tc:
        with ExitStack() as ctx:
            _emit(ctx, tc, d)
    nc.compile()
    return nc


def get_nc():
    if "nc" not in _CACHE:
        _CACHE["nc"] = _build()
    return _CACHE["nc"]


def make_in_maps(inputs):
    x = np.asarray(inputs["hidden_states"], dtype=np.float32)
    wq = np.asarray(inputs["wq"], dtype=np.float32)
    wk = np.asarray(inputs["wk"], dtype=np.float32)
    wv = np.asarray(inputs["wv"], dtype=np.float32)
    wo = np.asarray(inputs["wo"], dtype=np.float32)
    bq = np.asarray(inputs["bq"], dtype=np.float32)
    bk = np.asarray(inputs["bk"], dtype=np.float32)
    bv = np.asarray(inputs["bv"], dtype=np.float32)

    wqT = np.ascontiguousarray(wq.T).astype(BFNP)
    wkT = np.ascontiguousarray(wk.T).astype(BFNP)
    wvT = np.ascontiguousarray(wv.T).astype(BFNP)
    woT = np.ascontiguousarray(wo.T).astype(BFNP)
    bvrow = bv.reshape(1, H).astype(BFNP)
    # feature-major bias chunks: partition p, col fc -> bias[fc*128 + p]
    bqc = np.ascontiguousarray(bq.reshape(FC, 128).T)
    bkc = np.ascontiguousarray(bk.reshape(FC, 128).T)

    in_maps = []
    for c in range(NCORES):
        b, hf = divmod(c, 2)
        xb = x[b]
        # roll so this core's query tokens are tokens [0:T); key order is
        # irrelevant to attention (softmax/PV sum over keys).
        rolled = np.concatenate([xb[hf * T:], xb[:hf * T]], axis=0) if hf else xb
        xT = np.ascontiguousarray(rolled.T).astype(BFNP)
        in_maps.append({
            "xT": xT, "wqT": wqT, "wkT": wkT, "wvT": wvT,
            "bvrow": bvrow, "woT": woT, "bqc": bqc, "bkc": bkc,
        })
    return in_maps


def kernel(**inputs):
    nc = get_nc()
    in_maps = make_in_maps(inputs)
    res = run_bass_kernel_spmd(nc, in_maps, core_ids=list(range(NCORES)))
    bo = np.asarray(inputs["bo"], dtype=np.float32)
    out = np.empty((B, S, H), dtype=np.float32)
    for c in range(NCORES):
        b, hf = divmod(c, 2)
        out[b, hf * T:(hf + 1) * T, :] = res.results[c]["out"]
    out += bo[None, None, :]
    return out


# revision 8
# speedup vs baseline: 2.1342x; 1.2024x over previous
"""Multi-head attention (B=4, S=2048, H=1024, 16 heads) on 8 Trainium2 NeuronCores.

Sharding: data-parallel over (batch, seq-half) -> 8 fully independent shards, no
collectives. Each core computes attention for 1024 query tokens of one batch
element; K/V are computed redundantly for the full 2048-token sequence of that
batch (+25% projection flops buys zero cross-core communication).

Per-core scheme (feature-major "transposed" layout), software-pipelined so the
ACT engine (exp) starts while the PE is still doing projections:

  emit order: V projection -> [qk(0); for hp: ST/exp(hp); qk(hp+1); PV(hp)]
              -> output projection

  - qT/kT feature-major [H, tokens]; each 128-row chunk = one head pair
  - scores transposed S_T[j,i]; both heads of a pair write ONE [128,1024]
    PSUM tile ([h0 512i | h1 512i]) via tile_position row packing -> the
    paired matmuls issue back-to-back and overlap on the PE (K=64 solo
    streams at half rate; pairing measured 143ns vs 492ns per mm)
  - softmax: one exp per [128,1024] tile on ACT ((N+352)/1.2 ns); no max
    subtraction needed (|s|/8 < 8 << 88); denominators ride along as a 65th
    ones-column of v in the PV matmul
  - PV: stationary = probs chunk, moving = v_ext; all 8 accumulation chains
    (4 ic x 2 heads) of an (hp,ih) live in ONE [128,776] PSUM tile: cols
    0..519 hold the 8 o_ext chains, cols 520..775 take the PE-transposed
    (feature-major) outputs via a bf16 bitcast view -> probs tiles are
    consumed jc-at-a-time (prb pool can be tiny -> SBUF fits the overlap)
  - divide by denominator: reciprocal + per-partition tensor_scalar on DVE
  - output projection accumulates all 16 heads in PSUM (contraction over
    feature chunks = head pairs)
  - biases: bq/bk per-partition adds on the PSUM->SBUF copy (DVE); bv via an
    augmented ones-row matmul; bo added on host (exact, linear).
"""

import numpy as np
import ml_dtypes
from contextlib import ExitStack

import concourse.tile as tile
from concourse import bacc, mybir
from concourse.bass_utils import run_bass_kernel_spmd
from concourse.masks import make_identity

B, S, H, NH, HD = 4, 2048, 1024, 16, 64
T = 1024          # query tokens per core
TK = 2048         # key tokens per core
NCORES = 8
FC = H // 128     # 8 feature chunks (= head pairs)
KC = TK // 128    # 16 key-token chunks
F32 = mybir.dt.float32
BF16 = mybir.dt.bfloat16
BFNP = ml_dtypes.bfloat16
Act = mybir.ActivationFunctionType
Alu = mybir.AluOpType

# po tile layout (f32 cols, [128,1024] = 2 PSUM banks; no accumulation chain
# crosses a 2KB zero-region boundary):
#   bank0: chains ic0/ic1/ic2 at 0/130/260 (65 cols per head), tr0 at 390
#   bank1: chain ic3 at 512, tr1/tr2/tr3 at 642/706/770
CHAIN_OFF = [0, 130, 260, 512]
TR_OFF = [390, 642, 706, 770]
POW = 1024

_CACHE = {}


def _emit(ctx: ExitStack, tc, d):
    nc = tc.nc
    xT, wqT, wkT, wvT = d["xT"], d["wqT"], d["wkT"], d["wvT"]
    bvrow, woT, bqc, bkc, out = d["bvrow"], d["woT"], d["bqc"], d["bkc"], d["out"]

    pers = ctx.enter_context(tc.tile_pool(name="pers", bufs=1))
    ident = pers.tile([128, 128], BF16, tag="ident")
    make_identity(nc, ident[:])

    kT_t = [pers.tile([128, TK], BF16, tag="kT", bufs=FC, name=f"kT{i}") for i in range(FC)]
    qT_t = [pers.tile([128, T], BF16, tag="qT", bufs=FC, name=f"qT{i}") for i in range(FC)]
    v_t = [pers.tile([128, NH * (HD + 1)], BF16, tag="vsb", bufs=KC, name=f"v{i}") for i in range(KC)]
    oT_t = [pers.tile([128, T], BF16, tag="oT", bufs=FC, name=f"oT{i}") for i in range(FC)]
    bq_sb = pers.tile([128, FC], F32, tag="bq")
    bk_sb = pers.tile([128, FC], F32, tag="bk")
    nc.sync.dma_start(bq_sb[:], bqc[:, :])
    nc.sync.dma_start(bk_sb[:], bkc[:, :])

    # attention working pools, allocated below the ph1 input pools so their
    # slots don't depend on ph1 frees (enables ph1/attention overlap)
    attn = ctx.enter_context(tc.tile_pool(name="attn", bufs=1))
    # PSUM stack: pst(4) + po(2) live for the whole attention; ppool(2) on
    # top, released after the last qk chain; pf(2) then reuses its banks.
    pst = ctx.enter_context(tc.tile_pool(name="pst", bufs=2, space="PSUM"))
    po = ctx.enter_context(tc.tile_pool(name="po", bufs=1, space="PSUM"))

    stack_x = ExitStack()
    stack_qk = ExitStack()
    stack_v = ExitStack()

    pin = stack_x.enter_context(tc.tile_pool(name="pin", bufs=1))
    x_t = [pin.tile([128, TK], BF16, tag="xin", bufs=FC, name=f"x{i}") for i in range(FC)]
    ones_r = pin.tile([1, 128], BF16, tag="ones")
    pqk = stack_qk.enter_context(tc.tile_pool(name="pqk", bufs=1))
    wq_t = [pqk.tile([128, H], BF16, tag="wqin", bufs=FC, name=f"wq{i}") for i in range(FC)]
    wk_t = [pqk.tile([128, H], BF16, tag="wkin", bufs=FC, name=f"wk{i}") for i in range(FC)]
    pv_in = stack_v.enter_context(tc.tile_pool(name="pvin", bufs=1))
    wv_t = [pv_in.tile([128, H], BF16, tag="wvin", bufs=FC, name=f"wv{i}") for i in range(FC)]
    bv_r = pv_in.tile([1, H], BF16, tag="bvr")

    for fc in range(FC):
        nc.sync.dma_start(x_t[fc][:], xT[fc * 128:(fc + 1) * 128, :])
        nc.sync.dma_start(wv_t[fc][:], wvT[fc * 128:(fc + 1) * 128, :])
    for fc in range(FC):
        nc.sync.dma_start(wq_t[fc][:], wqT[fc * 128:(fc + 1) * 128, :])
        nc.sync.dma_start(wk_t[fc][:], wkT[fc * 128:(fc + 1) * 128, :])
    nc.sync.dma_start(bv_r[:1, :], bvrow[:, :])
    nc.gpsimd.memset(ones_r[:1, :], 1.0)

    pp = ExitStack()
    ppool = pp.enter_context(tc.tile_pool(name="ppool", bufs=2, space="PSUM"))

    # ---- V projection, token-major (+bias via ones-row) ----
    for kc in range(KC):
        nc.gpsimd.memset(v_t[kc][:], 1.0)  # ones cols survive at 65*h+64
        for mh in range(H // 512):
            ps = ppool.tile([128, 512], F32, tag="pp", name=f"psv{kc}_{mh}")
            for c in range(FC):
                nc.tensor.matmul(
                    ps[:], lhsT=x_t[c][:, kc * 128:(kc + 1) * 128],
                    rhs=wv_t[c][:, mh * 512:(mh + 1) * 512],
                    start=(c == 0), stop=False)
            nc.tensor.matmul(
                ps[:], lhsT=ones_r[:1, :],
                rhs=bv_r[:1, mh * 512:(mh + 1) * 512],
                start=False, stop=True)
            for hh in range(512 // HD):
                h = mh * 8 + hh
                nc.vector.tensor_copy(
                    v_t[kc][:, h * (HD + 1):h * (HD + 1) + HD],
                    ps[:, hh * HD:(hh + 1) * HD])
    stack_v.close()

    def emit_qk(fc):
        for th in range(T // 512):
            ps = ppool.tile([128, 512], F32, tag="pp", name=f"psq{fc}_{th}")
            for c in range(FC):
                nc.tensor.matmul(
                    ps[:], lhsT=wq_t[c][:, fc * 128:(fc + 1) * 128],
                    rhs=x_t[c][:, th * 512:(th + 1) * 512],
                    start=(c == 0), stop=(c == FC - 1))
            nc.vector.tensor_scalar(qT_t[fc][:, th * 512:(th + 1) * 512], ps[:],
                                    bq_sb[:, fc:fc + 1], None, Alu.add)
        for th in range(TK // 512):
            ps = ppool.tile([128, 512], F32, tag="pp", name=f"psk{fc}_{th}")
            for c in range(FC):
                nc.tensor.matmul(
                    ps[:], lhsT=wk_t[c][:, fc * 128:(fc + 1) * 128],
                    rhs=x_t[c][:, th * 512:(th + 1) * 512],
                    start=(c == 0), stop=(c == FC - 1))
            nc.vector.tensor_scalar(kT_t[fc][:, th * 512:(th + 1) * 512], ps[:],
                                    bk_sb[:, fc:fc + 1], None, Alu.add)

    prb = attn  # probs tiles come from the attn pool (small, consumed jc-wise)

    def emit_st_exp(hp, ih):
        i0 = ih * 512
        ptiles = [None] * KC
        for jc in range(KC):
            st2 = pst.tile([128, 1024], F32, tag="st", name=f"st{hp}_{ih}_{jc}")
            nc.tensor.matmul(
                st2[:, 0:512],
                lhsT=kT_t[hp][0:64, jc * 128:(jc + 1) * 128],
                rhs=qT_t[hp][0:64, i0:i0 + 512],
                start=True, stop=True, tile_position=(0, 0))
            nc.tensor.matmul(
                st2[:, 512:1024],
                lhsT=kT_t[hp][64:128, jc * 128:(jc + 1) * 128],
                rhs=qT_t[hp][64:128, i0:i0 + 512],
                start=True, stop=True, tile_position=(64, 0))
            pr = prb.tile([128, 1024], BF16, tag="pr", bufs=6,
                          name=f"pr{hp}_{ih}_{jc}")
            nc.scalar.activation(pr[:], st2[:], Act.Exp, scale=0.125)
            ptiles[jc] = pr
        return ptiles

    def emit_pv(hp, ih, ptiles):
        i0 = ih * 512
        oe = po.tile([128, POW], F32, tag="po", name=f"oe{hp}_{ih}")
        # jc-outer so each probs tile is consumed by its 8 matmuls and freed.
        # One start per 2KB zero region (it marks the whole region pending-
        # zero, so the other chains' first writes overwrite correctly with
        # start=False); explicit ordering deps keep the start matmul first.
        bank_start = {}
        for jc in range(KC):
            for ic in range(4):
                for hh in range(2):
                    h = 2 * hp + hh
                    ob = CHAIN_OFF[ic] + hh * (HD + 1)
                    bank = 0 if ic < 3 else 1
                    is_start = jc == 0 and bank not in bank_start
                    is_stop = jc == KC - 1 and (
                        (bank == 0 and ic == 2 and hh == 1) or (bank == 1 and hh == 1))
                    mm = nc.tensor.matmul(
                        oe[:, ob:ob + HD + 1],
                        lhsT=ptiles[jc][:, hh * 512 + ic * 128:hh * 512 + ic * 128 + 128],
                        rhs=v_t[jc][:, h * (HD + 1):(h + 1) * (HD + 1)],
                        start=is_start, stop=is_stop, skip_group_check=True)
                    if is_start:
                        bank_start[bank] = mm.ins
                    elif jc == 0:
                        tile.add_dep_helper(mm.ins, bank_start[bank], sync=False,
                                            reason="psum zero-region start first")
        for ic in range(4):
            trr = oe[:, TR_OFF[ic]:TR_OFF[ic] + 64].bitcast(BF16)
            for hh in range(2):
                ob = CHAIN_OFF[ic] + hh * (HD + 1)
                rec = attn.tile([128, 1], F32, tag="rec", bufs=4, name=f"rec{hp}_{ih}_{ic}_{hh}")
                nc.vector.reciprocal(rec[:], oe[:, ob + HD:ob + HD + 1])
                od = attn.tile([128, HD], BF16, tag="od", bufs=4, name=f"od{hp}_{ih}_{ic}_{hh}")
                nc.vector.tensor_scalar(od[:], oe[:, ob:ob + HD], rec[:], None, Alu.mult)
                nc.tensor.transpose(trr[hh * 64:(hh + 1) * 64, :], od[:],
                                    ident[:], tile_position=(0, hh * 64))
            nc.vector.tensor_copy(
                oT_t[hp][:, i0 + ic * 128:i0 + (ic + 1) * 128], trr[:])

    # ---- software-pipelined qk + attention ----
    emit_qk(0)
    for hp in range(FC):
        pts = [emit_st_exp(hp, ih) for ih in range(T // 512)]
        if hp + 1 < FC:
            emit_qk(hp + 1)
        for ih in range(T // 512):
            emit_pv(hp, ih, pts[ih])
    stack_qk.close()
    stack_x.close()
    pp.close()

    # ---- output projection (woT loads into space freed by input pools) ----
    with ExitStack() as ph3:
        pwo = ph3.enter_context(tc.tile_pool(name="pwo", bufs=1))
        pf = ph3.enter_context(tc.tile_pool(name="pf", bufs=2, space="PSUM"))
        wo_t = [pwo.tile([128, H], BF16, tag="woT", bufs=FC, name=f"wo{i}") for i in range(FC)]
        for fc in range(FC):
            nc.sync.dma_start(wo_t[fc][:], woT[fc * 128:(fc + 1) * 128, :])
        for tcn in range(T // 128):
            fo = attn.tile([128, H], F32, tag="fo", bufs=2, name=f"fo{tcn}")
            for mh in range(H // 512):
                psf = pf.tile([128, 512], F32, tag="pf", name=f"pf{tcn}_{mh}")
                for fc in range(FC):
                    nc.tensor.matmul(
                        psf[:], lhsT=oT_t[fc][:, tcn * 128:(tcn + 1) * 128],
                        rhs=wo_t[fc][:, mh * 512:(mh + 1) * 512],
                        start=(fc == 0), stop=(fc == FC - 1))
                nc.vector.tensor_copy(fo[:, mh * 512:(mh + 1) * 512], psf[:])
            nc.sync.dma_start(out[tcn * 128:(tcn + 1) * 128, :], fo[:])


def _build():
    nc = bacc.Bacc("TRN2", target_bir_lowering=False, debug=False, enable_asserts=True)
    d = {}
    d["xT"] = nc.dram_tensor("xT", [H, TK], BF16, kind="ExternalInput").ap()
    d["wqT"] = nc.dram_tensor("wqT", [H, H], BF16, kind="ExternalInput").ap()
    d["wkT"] = nc.dram_tensor("wkT", [H, H], BF16, kind="ExternalInput").ap()
    d["wvT"] = nc.dram_tensor("wvT", [H, H], BF16, kind="ExternalInput").ap()
    d["bvrow"] = nc.dram_tensor("bvrow", [1, H], BF16, kind="ExternalInput").ap()
    d["woT"] = nc.dram_tensor("woT", [H, H], BF16, kind="ExternalInput").ap()
    d["bqc"] = nc.dram_tensor("bqc", [128, FC], F32, kind="ExternalInput").ap()
    d["bkc"] = nc.dram_tensor("bkc", [128, FC], F32, kind="ExternalInput").ap()
    d["out"] = nc.dram_tensor("out", [T, H], F32, kind="ExternalOutput").ap()
    with tile.TileContext(nc) as tc:
        with ExitStack() as ctx:
            _emit(ctx, tc, d)
    nc.compile()
    return nc


def get_nc():
    if "nc" not in _CACHE:
        _CACHE["nc"] = _build()
    return _CACHE["nc"]


def make_in_maps(inputs):
    x = np.asarray(inputs["hidden_states"], dtype=np.float32)
    wq = np.asarray(inputs["wq"], dtype=np.float32)
    wk = np.asarray(inputs["wk"], dtype=np.float32)
    wv = np.asarray(inputs["wv"], dtype=np.float32)
    wo = np.asarray(inputs["wo"], dtype=np.float32)
    bq = np.asarray(inputs["bq"], dtype=np.float32)
    bk = np.asarray(inputs["bk"], dtype=np.float32)
    bv = np.asarray(inputs["bv"], dtype=np.float32)

    wqT = np.ascontiguousarray(wq.T).astype(BFNP)
    wkT = np.ascontiguousarray(wk.T).astype(BFNP)
    wvT = np.ascontiguousarray(wv.T).astype(BFNP)
    woT = np.ascontiguousarray(wo.T).astype(BFNP)
    bvrow = bv.reshape(1, H).astype(BFNP)
    # feature-major bias chunks: partition p, col fc -> bias[fc*128 + p]
    bqc = np.ascontiguousarray(bq.reshape(FC, 128).T)
    bkc = np.ascontiguousarray(bk.reshape(FC, 128).T)

    in_maps = []
    for c in range(NCORES):
        b, hf = divmod(c, 2)
        xb = x[b]
        # roll so this core's query tokens are tokens [0:T); key order is
        # irrelevant to attention (softmax/PV sum over keys).
        rolled = np.concatenate([xb[hf * T:], xb[:hf * T]], axis=0) if hf else xb
        xT = np.ascontiguousarray(rolled.T).astype(BFNP)
        in_maps.append({
            "xT": xT, "wqT": wqT, "wkT": wkT, "wvT": wvT,
            "bvrow": bvrow, "woT": woT, "bqc": bqc, "bkc": bkc,
        })
    return in_maps


def kernel(**inputs):
    nc = get_nc()
    in_maps = make_in_maps(inputs)
    res = run_bass_kernel_spmd(nc, in_maps, core_ids=list(range(NCORES)))
    bo = np.asarray(inputs["bo"], dtype=np.float32)
    out = np.empty((B, S, H), dtype=np.float32)
    for c in range(NCORES):
        b, hf = divmod(c, 2)
        out[b, hf * T:(hf + 1) * T, :] = res.results[c]["out"]
    out += bo[None, None, :]
    return out


# revision 10
# speedup vs baseline: 2.2334x; 1.0465x over previous
"""Multi-head attention (B=4, S=2048, H=1024, 16 heads) on 8 Trainium2 NeuronCores.

Sharding: data-parallel over (batch, seq-half) -> 8 fully independent shards, no
collectives. Each core computes attention for 1024 query tokens of one batch
element; K/V are computed redundantly for the full 2048-token sequence of that
batch (+25% projection flops buys zero cross-core communication).

Per-core scheme (feature-major "transposed" layout), software-pipelined so the
ACT engine (exp) starts while the PE is still doing projections:

  emit order: V projection -> [qk(0); for hp: ST/exp(hp); qk(hp+1); PV(hp)]
              -> output projection

  - qT/kT feature-major [H, tokens]; each 128-row chunk = one head pair
  - scores transposed S_T[j,i]; both heads of a pair write ONE [128,1024]
    PSUM tile ([h0 512i | h1 512i]) via tile_position row packing -> the
    paired matmuls issue back-to-back and overlap on the PE (K=64 solo
    streams at half rate; pairing measured 143ns vs 492ns per mm)
  - softmax: one exp per [128,1024] tile on ACT ((N+352)/1.2 ns); no max
    subtraction needed (|s|/8 < 8 << 88); denominators ride along as a 65th
    ones-column of v in the PV matmul
  - PV: stationary = probs chunk, moving = v_ext; all 8 accumulation chains
    (4 ic x 2 heads) of an (hp,ih) live in ONE [128,776] PSUM tile: cols
    0..519 hold the 8 o_ext chains, cols 520..775 take the PE-transposed
    (feature-major) outputs via a bf16 bitcast view -> probs tiles are
    consumed jc-at-a-time (prb pool can be tiny -> SBUF fits the overlap)
  - divide by denominator: reciprocal + per-partition tensor_scalar on DVE
  - output projection accumulates all 16 heads in PSUM (contraction over
    feature chunks = head pairs)
  - biases: bq/bk per-partition adds on the PSUM->SBUF copy (DVE); bv via an
    augmented ones-row matmul; bo added on host (exact, linear).
"""

import numpy as np
import ml_dtypes
from contextlib import ExitStack

import concourse.tile as tile
from concourse import bacc, mybir
from concourse.bass_utils import run_bass_kernel_spmd
from concourse.masks import make_identity

B, S, H, NH, HD = 4, 2048, 1024, 16, 64
T = 1024          # query tokens per core
TK = 2048         # key tokens per core
NCORES = 8
FC = H // 128     # 8 feature chunks (= head pairs)
KC = TK // 128    # 16 key-token chunks
F32 = mybir.dt.float32
BF16 = mybir.dt.bfloat16
BFNP = ml_dtypes.bfloat16
Act = mybir.ActivationFunctionType
Alu = mybir.AluOpType

# po tile layout (f32 cols, [128,1024] = 2 PSUM banks; no accumulation chain
# crosses a 2KB zero-region boundary):
#   bank0: chains ic0/ic1/ic2 at 0/130/260 (65 cols per head), tr0 at 390
#   bank1: chain ic3 at 512, tr1/tr2/tr3 at 642/706/770
CHAIN_OFF = [0, 130, 260, 512]
TR_OFF = [390, 642, 706, 770]
POW = 1024

_CACHE = {}


def _emit(ctx: ExitStack, tc, d):
    nc = tc.nc
    xT, wqT, wkT, wvT = d["xT"], d["wqT"], d["wkT"], d["wvT"]
    bvrow, woT, bqc, bkc, out = d["bvrow"], d["woT"], d["bqc"], d["bkc"], d["out"]

    pers = ctx.enter_context(tc.tile_pool(name="pers", bufs=1))
    ident = pers.tile([128, 128], BF16, tag="ident")
    make_identity(nc, ident[:])

    kT_t = [pers.tile([128, TK], BF16, tag="kT", bufs=FC, name=f"kT{i}") for i in range(FC)]
    qT_t = [pers.tile([128, T], BF16, tag="qT", bufs=FC, name=f"qT{i}") for i in range(FC)]
    v_t = [pers.tile([128, NH * (HD + 1)], BF16, tag="vsb", bufs=KC, name=f"v{i}") for i in range(KC)]
    oT_t = [pers.tile([128, T], BF16, tag="oT", bufs=FC, name=f"oT{i}") for i in range(FC)]
    bq_sb = pers.tile([128, FC], F32, tag="bq")
    bk_sb = pers.tile([128, FC], F32, tag="bk")
    nc.sync.dma_start(bq_sb[:], bqc[:, :])
    nc.sync.dma_start(bk_sb[:], bkc[:, :])

    # attention working pools, allocated below the ph1 input pools so their
    # slots don't depend on ph1 frees (enables ph1/attention overlap)
    attn = ctx.enter_context(tc.tile_pool(name="attn", bufs=1))
    # PSUM stack: pst(4) + po(2) live for the whole attention; ppool(2) on
    # top, released after the last qk chain; pf(2) then reuses its banks.
    pst = ctx.enter_context(tc.tile_pool(name="pst", bufs=2, space="PSUM"))
    po = ctx.enter_context(tc.tile_pool(name="po", bufs=1, space="PSUM"))

    stack_x = ExitStack()
    stack_qk = ExitStack()
    stack_v = ExitStack()

    pin = stack_x.enter_context(tc.tile_pool(name="pin", bufs=1))
    x_t = [pin.tile([128, TK], BF16, tag="xin", bufs=FC, name=f"x{i}") for i in range(FC)]
    ones_r = pin.tile([1, 128], BF16, tag="ones")
    pqk = stack_qk.enter_context(tc.tile_pool(name="pqk", bufs=1))
    wq_t = [pqk.tile([128, H], BF16, tag="wqin", bufs=FC, name=f"wq{i}") for i in range(FC)]
    wk_t = [pqk.tile([128, H], BF16, tag="wkin", bufs=FC, name=f"wk{i}") for i in range(FC)]
    pv_in = stack_v.enter_context(tc.tile_pool(name="pvin", bufs=1))
    wv_t = [pv_in.tile([128, H], BF16, tag="wvin", bufs=FC, name=f"wv{i}") for i in range(FC)]
    bv_r = pv_in.tile([1, H], BF16, tag="bvr")

    for fc in range(FC):
        # column-split so the first v chains unblock after half the x bytes
        nc.sync.dma_start(x_t[fc][:, 0:1024], xT[fc * 128:(fc + 1) * 128, 0:1024])
        nc.sync.dma_start(wv_t[fc][:], wvT[fc * 128:(fc + 1) * 128, :])
    for fc in range(FC):
        nc.sync.dma_start(x_t[fc][:, 1024:2048], xT[fc * 128:(fc + 1) * 128, 1024:2048])
    for fc in range(FC):
        nc.sync.dma_start(wq_t[fc][:], wqT[fc * 128:(fc + 1) * 128, :])
        nc.sync.dma_start(wk_t[fc][:], wkT[fc * 128:(fc + 1) * 128, :])
    nc.sync.dma_start(bv_r[:1, :], bvrow[:, :])
    nc.gpsimd.memset(ones_r[:1, :], 1.0)

    pp = ExitStack()
    ppool = pp.enter_context(tc.tile_pool(name="ppool", bufs=2, space="PSUM"))

    # ---- V projection, token-major (+bias via ones-row) ----
    for kc in range(KC):
        nc.gpsimd.memset(v_t[kc][:], 1.0)  # ones cols survive at 65*h+64
        for mh in range(H // 512):
            ps = ppool.tile([128, 512], F32, tag="pp", name=f"psv{kc}_{mh}")
            for c in range(FC):
                nc.tensor.matmul(
                    ps[:], lhsT=x_t[c][:, kc * 128:(kc + 1) * 128],
                    rhs=wv_t[c][:, mh * 512:(mh + 1) * 512],
                    start=(c == 0), stop=False)
            nc.tensor.matmul(
                ps[:], lhsT=ones_r[:1, :],
                rhs=bv_r[:1, mh * 512:(mh + 1) * 512],
                start=False, stop=True)
            for hh in range(512 // HD):
                h = mh * 8 + hh
                nc.vector.tensor_copy(
                    v_t[kc][:, h * (HD + 1):h * (HD + 1) + HD],
                    ps[:, hh * HD:(hh + 1) * HD])
    stack_v.close()

    def emit_qk(fc):
        for th in range(T // 512):
            ps = ppool.tile([128, 512], F32, tag="pp", name=f"psq{fc}_{th}")
            for c in range(FC):
                nc.tensor.matmul(
                    ps[:], lhsT=wq_t[c][:, fc * 128:(fc + 1) * 128],
                    rhs=x_t[c][:, th * 512:(th + 1) * 512],
                    start=(c == 0), stop=(c == FC - 1))
            nc.vector.tensor_scalar(qT_t[fc][:, th * 512:(th + 1) * 512], ps[:],
                                    bq_sb[:, fc:fc + 1], None, Alu.add)
        for th in range(TK // 512):
            ps = ppool.tile([128, 512], F32, tag="pp", name=f"psk{fc}_{th}")
            for c in range(FC):
                nc.tensor.matmul(
                    ps[:], lhsT=wk_t[c][:, fc * 128:(fc + 1) * 128],
                    rhs=x_t[c][:, th * 512:(th + 1) * 512],
                    start=(c == 0), stop=(c == FC - 1))
            nc.vector.tensor_scalar(kT_t[fc][:, th * 512:(th + 1) * 512], ps[:],
                                    bk_sb[:, fc:fc + 1], None, Alu.add)

    prb = attn  # probs tiles come from the attn pool (small, consumed jc-wise)

    def emit_st_exp(hp, ih):
        i0 = ih * 512
        ptiles = [None] * KC
        for jc in range(KC):
            st2 = pst.tile([128, 1024], F32, tag="st", name=f"st{hp}_{ih}_{jc}")
            nc.tensor.matmul(
                st2[:, 0:512],
                lhsT=kT_t[hp][0:64, jc * 128:(jc + 1) * 128],
                rhs=qT_t[hp][0:64, i0:i0 + 512],
                start=True, stop=True, tile_position=(0, 0))
            nc.tensor.matmul(
                st2[:, 512:1024],
                lhsT=kT_t[hp][64:128, jc * 128:(jc + 1) * 128],
                rhs=qT_t[hp][64:128, i0:i0 + 512],
                start=True, stop=True, tile_position=(64, 0))
            pr = prb.tile([128, 1024], BF16, tag="pr", bufs=6,
                          name=f"pr{hp}_{ih}_{jc}")
            nc.scalar.activation(pr[:], st2[:], Act.Exp, scale=0.125)
            ptiles[jc] = pr
        return ptiles

    def emit_pv(hp, ih, ptiles):
        i0 = ih * 512
        oe = po.tile([128, POW], F32, tag="po", name=f"oe{hp}_{ih}")
        # jc-outer so each probs tile is consumed by its 8 matmuls and freed.
        # One start per 2KB zero region (it marks the whole region pending-
        # zero, so the other chains' first writes overwrite correctly with
        # start=False); explicit ordering deps keep the start matmul first.
        bank_start = {}
        for jc in range(KC):
            for ic in range(4):
                for hh in range(2):
                    h = 2 * hp + hh
                    ob = CHAIN_OFF[ic] + hh * (HD + 1)
                    bank = 0 if ic < 3 else 1
                    is_start = jc == 0 and bank not in bank_start
                    is_stop = jc == KC - 1 and (
                        (bank == 0 and ic == 2 and hh == 1) or (bank == 1 and hh == 1))
                    mm = nc.tensor.matmul(
                        oe[:, ob:ob + HD + 1],
                        lhsT=ptiles[jc][:, hh * 512 + ic * 128:hh * 512 + ic * 128 + 128],
                        rhs=v_t[jc][:, h * (HD + 1):(h + 1) * (HD + 1)],
                        start=is_start, stop=is_stop, skip_group_check=True)
                    if is_start:
                        bank_start[bank] = mm.ins
                    elif jc == 0:
                        tile.add_dep_helper(mm.ins, bank_start[bank], sync=False,
                                            reason="psum zero-region start first")
        for ic in range(4):
            trr = oe[:, TR_OFF[ic]:TR_OFF[ic] + 64].bitcast(BF16)
            od2 = attn.tile([128, 2 * HD], BF16, tag="od", bufs=4, name=f"od{hp}_{ih}_{ic}")
            for hh in range(2):
                ob = CHAIN_OFF[ic] + hh * (HD + 1)
                rec = attn.tile([128, 1], F32, tag="rec", bufs=4, name=f"rec{hp}_{ih}_{ic}_{hh}")
                nc.vector.reciprocal(rec[:], oe[:, ob + HD:ob + HD + 1])
                nc.vector.tensor_scalar(od2[:, hh * HD:(hh + 1) * HD],
                                        oe[:, ob:ob + HD], rec[:], None, Alu.mult)
            # single [128,128] transpose: out row r = od2[:, r] -> rows 0..63
            # are head h0's features, 64..127 head h1's = the oT pair layout
            nc.tensor.transpose(trr[:], od2[:], ident[:])
            nc.vector.tensor_copy(
                oT_t[hp][:, i0 + ic * 128:i0 + (ic + 1) * 128], trr[:])

    # ---- software-pipelined qk + attention ----
    emit_qk(0)
    for hp in range(FC):
        pts = [emit_st_exp(hp, ih) for ih in range(T // 512)]
        if hp + 1 < FC:
            emit_qk(hp + 1)
        for ih in range(T // 512):
            emit_pv(hp, ih, pts[ih])
    stack_qk.close()
    stack_x.close()
    pp.close()

    # ---- output projection (woT loads into space freed by input pools) ----
    with ExitStack() as ph3:
        pwo = ph3.enter_context(tc.tile_pool(name="pwo", bufs=1))
        pf = ph3.enter_context(tc.tile_pool(name="pf", bufs=2, space="PSUM"))
        wo_t = [pwo.tile([128, H], BF16, tag="woT", bufs=FC, name=f"wo{i}") for i in range(FC)]
        for fc in range(FC):
            nc.sync.dma_start(wo_t[fc][:], woT[fc * 128:(fc + 1) * 128, :])
        for tcn in range(T // 128):
            fo = attn.tile([128, H], F32, tag="fo", bufs=2, name=f"fo{tcn}")
            for mh in range(H // 512):
                psf = pf.tile([128, 512], F32, tag="pf", name=f"pf{tcn}_{mh}")
                for fc in range(FC):
                    nc.tensor.matmul(
                        psf[:], lhsT=oT_t[fc][:, tcn * 128:(tcn + 1) * 128],
                        rhs=wo_t[fc][:, mh * 512:(mh + 1) * 512],
                        start=(fc == 0), stop=(fc == FC - 1))
                nc.vector.tensor_copy(fo[:, mh * 512:(mh + 1) * 512], psf[:])
            nc.sync.dma_start(out[tcn * 128:(tcn + 1) * 128, :], fo[:])


def _build():
    nc = bacc.Bacc("TRN2", target_bir_lowering=False, debug=False, enable_asserts=True)
    d = {}
    d["xT"] = nc.dram_tensor("xT", [H, TK], BF16, kind="ExternalInput").ap()
    d["wqT"] = nc.dram_tensor("wqT", [H, H], BF16, kind="ExternalInput").ap()
    d["wkT"] = nc.dram_tensor("wkT", [H, H], BF16, kind="ExternalInput").ap()
    d["wvT"] = nc.dram_tensor("wvT", [H, H], BF16, kind="ExternalInput").ap()
    d["bvrow"] = nc.dram_tensor("bvrow", [1, H], BF16, kind="ExternalInput").ap()
    d["woT"] = nc.dram_tensor("woT", [H, H], BF16, kind="ExternalInput").ap()
    d["bqc"] = nc.dram_tensor("bqc", [128, FC], F32, kind="ExternalInput").ap()
    d["bkc"] = nc.dram_tensor("bkc", [128, FC], F32, kind="ExternalInput").ap()
    d["out"] = nc.dram_tensor("out", [T, H], F32, kind="ExternalOutput").ap()
    with tile.TileContext(nc) as tc:
        with ExitStack() as ctx:
            _emit(ctx, tc, d)
    nc.compile()
    return nc


def get_nc():
    if "nc" not in _CACHE:
        _CACHE["nc"] = _build()
    return _CACHE["nc"]


def make_in_maps(inputs):
    x = np.asarray(inputs["hidden_states"], dtype=np.float32)
    wq = np.asarray(inputs["wq"], dtype=np.float32)
    wk = np.asarray(inputs["wk"], dtype=np.float32)
    wv = np.asarray(inputs["wv"], dtype=np.float32)
    wo = np.asarray(inputs["wo"], dtype=np.float32)
    bq = np.asarray(inputs["bq"], dtype=np.float32)
    bk = np.asarray(inputs["bk"], dtype=np.float32)
    bv = np.asarray(inputs["bv"], dtype=np.float32)

    wqT = np.ascontiguousarray(wq.T).astype(BFNP)
    wkT = np.ascontiguousarray(wk.T).astype(BFNP)
    wvT = np.ascontiguousarray(wv.T).astype(BFNP)
    woT = np.ascontiguousarray(wo.T).astype(BFNP)
    bvrow = bv.reshape(1, H).astype(BFNP)
    # feature-major bias chunks: partition p, col fc -> bias[fc*128 + p]
    bqc = np.ascontiguousarray(bq.reshape(FC, 128).T)
    bkc = np.ascontiguousarray(bk.reshape(FC, 128).T)

    in_maps = []
    for c in range(NCORES):
        b, hf = divmod(c, 2)
        xb = x[b]
        # roll so this core's query tokens are tokens [0:T); key order is
        # irrelevant to attention (softmax/PV sum over keys).
        rolled = np.concatenate([xb[hf * T:], xb[:hf * T]], axis=0) if hf else xb
        xT = np.ascontiguousarray(rolled.T).astype(BFNP)
        in_maps.append({
            "xT": xT, "wqT": wqT, "wkT": wkT, "wvT": wvT,
            "bvrow": bvrow, "woT": woT, "bqc": bqc, "bkc": bkc,
        })
    return in_maps


def kernel(**inputs):
    nc = get_nc()
    in_maps = make_in_maps(inputs)
    res = run_bass_kernel_spmd(nc, in_maps, core_ids=list(range(NCORES)))
    bo = np.asarray(inputs["bo"], dtype=np.float32)
    out = np.empty((B, S, H), dtype=np.float32)
    for c in range(NCORES):
        b, hf = divmod(c, 2)
        out[b, hf * T:(hf + 1) * T, :] = res.results[c]["out"]
    out += bo[None, None, :]
    return out
